# revision 21
# baseline (speedup 1.0000x reference)
"""Trainium2 Bass kernel for the sparse_attention PoC block.

Reference computation (per batch item):
  qkv = x @ qkv_w.T            [N, 3C] -> q,k,v heads [H, N, D]
  attn = (q @ k.T) * scale     [H, N, N]
  block edits: attn[:S1, S2:] = attn[:S1, S1:S2] (pre-bias copy), then
  -100 bias on [:S1, S1:S2], [S1:S2, S2:], [S2:, S1:S2]; softmax;
  attn @ v; proj.

Distribution: pure data-parallel over batch B=64 across 8 NeuronCores
(8 batch items per core, weights replicated). No collectives.

Schedule highlights (v2):
  - ALL transposes (weights at prep, x per batch) go through the DMA
    XBAR engine (dma_start_transpose, bf16), not the PE: out[p,c,n] =
    in[n, 128c+p].
  - qkv projection processes a PAIR of batch items per matmul (moving
    free dim 472) so LDWEIGHTS fully hides under the stream.
  - q,k produced in transposed per-head layout [96, H, 2, N] via
    fragment copies out of the 128-chunk PSUM (alternating DVE/ACT).
  - block-edit biases applied as per-partition bias vectors in the
    softmax exp ACTIVATE (3 column-range activations on the second key
    tile); contraction K stays 96 (no bias-extension rows).
  - the pre-bias "copy" edit rides as a free-dim overwrite of kT's
    aux-slot columns; a 20x20-per-head correction matmul restores the
    true aux x aux block (suppressed in the main tile by the bias).
  - softmax without max-subtraction; denominator via fused [v | ones]
    stationary column; normalize with fast reciprocal + gpsimd
    partition_broadcast.
  - v and proj stream in two head-aligned 384-column legs.

Partition-alignment rule (walrus verifier): compute-engine access
patterns must start at partition 0/32/64/96 (max 128/32/64/32
partitions); matmul operands must start at partition 0. Misaligned
extractions (v aux rows 88:108) go through DMA.
"""

import numpy as np

B, N, C = 64, 236, 768
H, D = 8, 96
S1, S2 = 196, 216
BIAS = 100.0
SCALE = D ** -0.5

N_CORES = 8
B_LOC = B // N_CORES

NT = [(0, 128), (128, 108)]  # token tiles (partition dim) / key tiles
NCH = C // 128  # 6 contraction chunks over C
NPAD = 240      # per-batch padded token width in xT (XBAR writes 112)
FL = [(0, 384), (384, 384)]  # head-aligned f-legs for v / proj


def part_cap(s):
    return 128 if s == 0 else 64 if s == 64 else 32


def part_pieces2(s1, s2, size):
    """Split a partition-range copy (dest start s1, src start s2, length
    size) into pieces legal for compute engines on both sides."""
    out = []
    off = 0
    while off < size:
        take = min(size - off, part_cap((s1 + off) % 128),
                   part_cap((s2 + off) % 128))
        out.append((s1 + off, s2 + off, take))
        off += take
    return out


def head_fragments(o_lo, o_hi, base):
    """Split channel range [o_lo, o_hi) (relative to `base`) at head
    boundaries (96) and legal partition pieces. Yields
    (head, d_lo, d_hi, p_lo, p_hi) with p relative to o_lo."""
    frags = []
    g = o_lo
    while g < o_hi:
        h = (g - base) // D
        d_lo = (g - base) - h * D
        take = min(o_hi - g, D - d_lo)
        for (d0, p0, sz) in part_pieces2(d_lo, g - o_lo, take):
            frags.append((h, d0, d0 + sz, p0, p0 + sz))
        g += take
    return frags




def build(b_loc=B_LOC):
    import concourse.bass as bass  # noqa: F401
    import concourse.tile as tile
    import concourse.bacc as bacc
    from concourse import mybir

    f32 = mybir.dt.float32
    bf16 = mybir.dt.bfloat16
    AF = mybir.ActivationFunctionType
    OP = mybir.AluOpType

    n_pair = b_loc // 2

    nc = bacc.Bacc("TRN2", target_bir_lowering=False)
    x_d = nc.dram_tensor("x", [b_loc, N, C], f32, kind="ExternalInput")
    qkvw_d = nc.dram_tensor("qkv_w", [3 * C, C], f32, kind="ExternalInput")
    projw_d = nc.dram_tensor("proj_w", [C, C], f32, kind="ExternalInput")
    projb_d = nc.dram_tensor("proj_b", [C], f32, kind="ExternalInput")
    out_d = nc.dram_tensor("out", [b_loc, N, C], f32, kind="ExternalOutput")

    with tile.TileContext(nc) as tc:
        with (
            tc.tile_pool(name="const", bufs=1) as constp,
            tc.tile_pool(name="wload", bufs=3) as wloadp,
            tc.tile_pool(name="xload", bufs=3) as xloadp,
            tc.tile_pool(name="xt", bufs=2) as xtp,
            tc.tile_pool(name="qk", bufs=2) as qkp,
            tc.tile_pool(name="vsb", bufs=2) as vsbp,
            tc.tile_pool(name="psb", bufs=2) as psbp,
            tc.tile_pool(name="ao", bufs=2) as aop,
            tc.tile_pool(name="osb", bufs=2) as osbp,
            tc.tile_pool(name="tiny", bufs=2) as tinyp,
            tc.tile_pool(name="ps_qk", bufs=2, space="PSUM") as ps_qk,
            tc.tile_pool(name="ps_s", bufs=2, space="PSUM") as ps_s,
            tc.tile_pool(name="ps_o", bufs=2, space="PSUM") as ps_o,
            tc.tile_pool(name="ps_m", bufs=2, space="PSUM") as ps_m,
        ):
            # ---------------- weights prep (no PE, all XBAR) -----------
            # qkvwT[p, c, oc] = qkv_w[oc, 128c+p]  (qkv_w.T chunk layout)
            qkvwT = constp.tile([128, NCH, 3 * C], bf16)
            projwT = constp.tile([128, NCH, C], bf16)
            for r in range(3 * C // 128):  # 18 row-chunks of qkv_w
                wl = wloadp.tile([128, C], f32, tag="wl")
                nc.sync.dma_start(wl[:], qkvw_d[r * 128:(r + 1) * 128, :])
                wb = wloadp.tile([128, C], bf16, tag="wb")
                nc.vector.tensor_copy(wb[:], wl[:])
                nc.sync.dma_start_transpose(
                    qkvwT[:, :, r * 128:(r + 1) * 128], wb[:])
            for r in range(C // 128):  # 6 row-chunks of proj_w
                wl = wloadp.tile([128, C], f32, tag="wl")
                nc.sync.dma_start(wl[:], projw_d[r * 128:(r + 1) * 128, :])
                wb = wloadp.tile([128, C], bf16, tag="wb")
                nc.vector.tensor_copy(wb[:], wl[:])
                nc.sync.dma_start_transpose(
                    projwT[:, :, r * 128:(r + 1) * 128], wb[:])
            # per-head re-layout for proj contraction (K=96):
            # projwTh[d, h, oc] = proj_w[oc, 96h+d]
            projwTh = constp.tile([96, H, C], bf16)
            for ci in range(NCH):
                for (h, d0, d1, p0, p1) in head_fragments(
                        ci * 128, (ci + 1) * 128, 0):
                    nc.vector.tensor_copy(projwTh[d0:d1, h, :],
                                          projwT[p0:p1, ci, :])

            # proj_b broadcast to [128, C]
            pb_row = constp.tile([1, C], f32)
            nc.sync.dma_start(pb_row[:], projb_d[None, :])
            pb_bcast = constp.tile([128, C], f32)
            nc.gpsimd.partition_broadcast(pb_bcast[:], pb_row[:])

            # bias vectors for the j-tile-1 exp epilogue, per query group
            # (rows are key indices 128+p): col0 img queries, col1 lang,
            # col2 aux. Built as a free-dim row, bounced through DRAM to
            # land per-partition (SBUF APs can't transpose).
            bv_row = constp.tile([1, 3, 128], f32)
            nc.vector.memset(bv_row[:], 0.0)
            nc.vector.memset(bv_row[0:1, 0, S1 - 128:S2 - 128], -BIAS)
            nc.vector.memset(bv_row[0:1, 1, S2 - 128:N - 128], -BIAS)
            nc.vector.memset(bv_row[0:1, 2, S1 - 128:N - 128], -BIAS)
            bv_d = nc.dram_tensor("bv_scratch", [3, 128], f32,
                                  kind="Internal")
            nc.sync.dma_start(bv_d[:, :], bv_row[0:1, :, :])
            bv = constp.tile([128, 3], f32)
            nc.sync.dma_start(bv[:, :], bv_d.rearrange("i p -> p i"))

            # ---------------- per-pair of batch items ----------------
            for pr in range(n_pair):
                # load + cast + XBAR-transpose x for both batch items.
                # XBARs dispatch from the scalar HWDGE queue so their
                # mode transitions never serialize the sync DMA queue.
                xT = xtp.tile([128, NCH, 2, NPAD], bf16, tag="xT")
                xbs = []
                for bi in range(2):
                    b = 2 * pr + bi
                    for nt, (noff, nsz) in enumerate(NT):
                        xf = xloadp.tile([128, C], f32, tag="xf")
                        nc.sync.dma_start(xf[:nsz],
                                          x_d[b, noff:noff + nsz, :])
                        xb = xloadp.tile([128, C], bf16, tag="xb")
                        if nt == 1:
                            nc.vector.memset(xb[96:128, :], 0.0)
                        nc.vector.tensor_copy(xb[:nsz], xf[:nsz])
                        xbs.append((bi, noff, nsz, xb))
                for (bi, noff, nsz, xb) in xbs:
                    rows = 128 if nsz == 128 else 112
                    nc.scalar.dma_start_transpose(
                        xT[:, :, bi, noff:noff + rows], xb[:rows, :])

                # q,k transposed per head [96, H, 2, N], pair-wide moving
                q_all = qkp.tile([96, H, 2, N], bf16, tag="q_all")
                k_all = qkp.tile([96, H, 2, N], bf16, tag="k_all")
                cp_i = 0
                for oi in range(2 * C // 128):  # 12 chunks of q,k channels
                    ps = ps_qk.tile([128, 2, N], f32, tag="qk")
                    for ci in range(NCH):
                        nc.tensor.matmul(
                            ps[:, :, :],
                            qkvwT[:, ci, oi * 128:(oi + 1) * 128],
                            xT[:, ci, :, 0:N],
                            start=(ci == 0), stop=(ci == NCH - 1))
                    t = (oi * 128) // C
                    dst = q_all if t == 0 else k_all
                    oq = oi - t * NCH
                    for (h, d0, d1, p0, p1) in head_fragments(
                            oq * 128, (oq + 1) * 128, 0):
                        if cp_i % 2 == 0:
                            nc.vector.tensor_copy(dst[d0:d1, h, :, :],
                                                  ps[p0:p1, :, :])
                        else:
                            nc.scalar.copy(dst[d0:d1, h, :, :],
                                           ps[p0:p1, :, :])
                        cp_i += 1
                # stash original aux-key vectors, then overwrite aux-slot
                # columns with lang key vectors (the pre-bias "copy" edit)
                k_aux = qkp.tile([96, H, 2, S2 - S1], bf16, tag="k_aux")
                nc.gpsimd.tensor_copy(k_aux[:], k_all[:, :, :, S2:N])
                nc.gpsimd.tensor_copy(k_all[:, :, :, S2:N],
                                      k_all[:, :, :, S1:S2])

                # v natural [n, H, D+1] per (batch, token-tile)
                vp = [[None, None], [None, None]]
                vap = [None, None]
                for bi in range(2):
                    for nt, (noff, nsz) in enumerate(NT):
                        vpt = vsbp.tile([128, H, D + 1], bf16,
                                        tag=f"vp{bi}{nt}")
                        for li, (f0, fsz) in enumerate(FL):
                            psv = ps_m.tile([128, 384], f32, tag="m")
                            for ci in range(NCH):
                                nc.tensor.matmul(
                                    psv[:nsz, :fsz],
                                    xT[:, ci, bi, noff:noff + nsz],
                                    qkvwT[:, ci,
                                          2 * C + f0:2 * C + f0 + fsz],
                                    start=(ci == 0), stop=(ci == NCH - 1))
                            nc.vector.tensor_copy(
                                vpt[:nsz, 4 * li:4 * li + 4, 0:D],
                                psv[:nsz, :fsz].rearrange(
                                    "p (h d) -> p h d", h=4))
                        nc.vector.memset(vpt[:nsz, :, D:D + 1], 1.0)
                        vp[bi][nt] = vpt
                    # aux-token v rows (tokens 216:236 = rows 88:108 of
                    # tile 1): misaligned for compute engines -> DMA
                    vat = vsbp.tile([S2 - S1, H, D + 1], bf16, tag="vap")
                    nc.sync.dma_start(vat[:], vp[bi][1][88:108, :, :])
                    vap[bi] = vat

                # attention, two heads at a time
                aoT = aop.tile([96, H, 2, N], bf16, tag="aoT")
                for bi in range(2):
                    for hp in range(H // 2):
                        h0 = 2 * hp
                        pe = []
                        for jt, (joff, jsz) in enumerate(NT):
                            psj = ps_s.tile([128, 2, N], f32, tag="s")
                            for hh in range(2):
                                nc.tensor.matmul(
                                    psj[:jsz, hh, :],
                                    k_all[:, h0 + hh, bi,
                                          joff:joff + jsz],
                                    q_all[:, h0 + hh, bi, :],
                                    start=True, stop=True,
                                    skip_group_check=True)
                            pet = psbp.tile([128, 2, N], bf16,
                                            tag=f"p{jt}")
                            if jt == 0:
                                nc.scalar.activation(
                                    pet[:jsz], psj[:jsz], AF.Exp,
                                    scale=SCALE)
                            else:
                                nc.scalar.activation(
                                    pet[:jsz, :, 0:S1],
                                    psj[:jsz, :, 0:S1], AF.Exp,
                                    scale=SCALE, bias=bv[:jsz, 0:1])
                                nc.scalar.activation(
                                    pet[:jsz, :, S1:S2],
                                    psj[:jsz, :, S1:S2], AF.Exp,
                                    scale=SCALE, bias=bv[:jsz, 1:2])
                                nc.scalar.activation(
                                    pet[:jsz, :, S2:N],
                                    psj[:jsz, :, S2:N], AF.Exp,
                                    scale=SCALE, bias=bv[:jsz, 2:3])
                            pe.append(pet)
                        # true aux x aux block (suppressed in main tile)
                        ps_aa = ps_s.tile([S2 - S1, 2, S2 - S1], f32,
                                          tag="s")
                        for hh in range(2):
                            nc.tensor.matmul(
                                ps_aa[:, hh, :],
                                k_aux[:, h0 + hh, bi, :],
                                q_all[:, h0 + hh, bi, S2:N],
                                start=True, stop=True,
                                skip_group_check=True)
                        p_aa = psbp.tile([S2 - S1, 2, S2 - S1], bf16,
                                         tag="paa")
                        nc.scalar.activation(p_aa[:], ps_aa[:], AF.Exp,
                                             scale=SCALE)
                        # attn @ [v | ones] -> oT [D+1, q] + denominator
                        pso = ps_o.tile([D + 1, 2, N], f32, tag="o")
                        for hh in range(2):
                            for jt, (joff, jsz) in enumerate(NT):
                                nc.tensor.matmul(
                                    pso[:, hh, :],
                                    vp[bi][jt][:jsz, h0 + hh, :],
                                    pe[jt][:jsz, hh, :],
                                    start=(jt == 0), stop=False,
                                    skip_group_check=True)
                            nc.tensor.matmul(
                                pso[:, hh, S2:N],
                                vap[bi][:, h0 + hh, :],
                                p_aa[:, hh, :],
                                start=False, stop=True,
                                skip_group_check=True)
                        # normalize both heads
                        den = tinyp.tile([1, 2, N], f32, tag="den")
                        nc.vector.tensor_copy(den[:], pso[D:D + 1, :, :])
                        r_f = tinyp.tile([1, 2, N], f32, tag="rf")
                        nc.vector.reciprocal_approx_fast(r_f[:], den[:])
                        rbc = tinyp.tile([128, 2, N], f32, tag="rbc")
                        nc.gpsimd.partition_broadcast(
                            rbc[:],
                            r_f[0:1, :, :].rearrange("p a b -> p (a b)"))
                        nc.vector.tensor_tensor(
                            aoT[:, h0:h0 + 2, bi, :], pso[0:D, :, :],
                            rbc[0:D, :, :], OP.mult)

                # proj + bias + store (contract per head, K=96)
                for bi in range(2):
                    b = 2 * pr + bi
                    for nt, (noff, nsz) in enumerate(NT):
                        osb = osbp.tile([128, C], f32, tag="osb")
                        for li, (f0, fsz) in enumerate(FL):
                            psp = ps_m.tile([128, 384], f32, tag="m")
                            for h in range(H):
                                nc.tensor.matmul(
                                    psp[:nsz, :fsz],
                                    aoT[:, h, bi, noff:noff + nsz],
                                    projwTh[:, h, f0:f0 + fsz],
                                    start=(h == 0), stop=(h == H - 1))
                            nc.vector.tensor_tensor(
                                osb[:nsz, f0:f0 + fsz], psp[:nsz, :fsz],
                                pb_bcast[:nsz, f0:f0 + fsz], OP.add)
                        nc.sync.dma_start(out_d[b, noff:noff + nsz, :],
                                          osb[:nsz])

    nc.compile()
    return nc


_NC_CACHE = {}


def _get_nc(b_loc):
    if b_loc not in _NC_CACHE:
        _NC_CACHE[b_loc] = build(b_loc)
    return _NC_CACHE[b_loc]


def _run(inputs, trace=False):
    from concourse.bass_utils import run_bass_kernel_spmd

    x = np.ascontiguousarray(np.asarray(inputs["x"], dtype=np.float32))
    qkv_w = np.ascontiguousarray(np.asarray(inputs["qkv_w"], dtype=np.float32))
    proj_w = np.ascontiguousarray(np.asarray(inputs["proj_w"], dtype=np.float32))
    proj_b = np.ascontiguousarray(np.asarray(inputs["proj_b"], dtype=np.float32))

    nc = _get_nc(B_LOC)
    in_maps = [
        {
            "x": np.ascontiguousarray(x[i * B_LOC:(i + 1) * B_LOC]),
            "qkv_w": qkv_w,
            "proj_w": proj_w,
            "proj_b": proj_b,
        }
        for i in range(N_CORES)
    ]
    res = run_bass_kernel_spmd(
        nc, in_maps, core_ids=list(range(N_CORES)), trace=trace)
    out = np.concatenate([r["out"] for r in res.results], axis=0)
    return out, res


def kernel(x, qkv_w, proj_w, proj_b):
    out, _ = _run({"x": x, "qkv_w": qkv_w, "proj_w": proj_w,
                   "proj_b": proj_b})
    return out


# revision 22
# speedup vs baseline: 1.2878x; 1.2878x over previous
"""Trainium2 Bass kernel for the sparse_attention PoC block.

Reference computation (per batch item):
  qkv = x @ qkv_w.T            [N, 3C] -> q,k,v heads [H, N, D]
  attn = (q @ k.T) * scale     [H, N, N]
  block edits: attn[:S1, S2:] = attn[:S1, S1:S2] (pre-bias copy), then
  -100 bias on [:S1, S1:S2], [S1:S2, S2:], [S2:, S1:S2]; softmax;
  attn @ v; proj.

Distribution: pure data-parallel over batch B=64 across 8 NeuronCores
(8 batch items per core, weights replicated). No collectives.

Schedule highlights (v2):
  - ALL transposes (weights at prep, x per batch) go through the DMA
    XBAR engine (dma_start_transpose, bf16), not the PE: out[p,c,n] =
    in[n, 128c+p].
  - qkv projection processes a PAIR of batch items per matmul (moving
    free dim 472) so LDWEIGHTS fully hides under the stream.
  - q,k produced in transposed per-head layout [96, H, 2, N] via
    fragment copies out of the 128-chunk PSUM (alternating DVE/ACT).
  - block-edit biases applied as per-partition bias vectors in the
    softmax exp ACTIVATE (3 column-range activations on the second key
    tile); contraction K stays 96 (no bias-extension rows).
  - the pre-bias "copy" edit rides as a free-dim overwrite of kT's
    aux-slot columns; a 20x20-per-head correction matmul restores the
    true aux x aux block (suppressed in the main tile by the bias).
  - softmax without max-subtraction; denominator via fused [v | ones]
    stationary column; normalize with fast reciprocal + gpsimd
    partition_broadcast.
  - v and proj stream in two head-aligned 384-column legs.

Partition-alignment rule (walrus verifier): compute-engine access
patterns must start at partition 0/32/64/96 (max 128/32/64/32
partitions); matmul operands must start at partition 0. Misaligned
extractions (v aux rows 88:108) go through DMA.
"""

import numpy as np

B, N, C = 64, 236, 768
H, D = 8, 96
S1, S2 = 196, 216
BIAS = 100.0
SCALE = D ** -0.5

N_CORES = 8
B_LOC = B // N_CORES

NT = [(0, 128), (128, 108)]  # token tiles (partition dim) / key tiles
NCH = C // 128  # 6 contraction chunks over C
NPAD = 240      # per-batch padded token width in xT (XBAR writes 112)
FL = [(0, 384), (384, 384)]  # head-aligned f-legs for v / proj


def part_cap(s):
    return 128 if s == 0 else 64 if s == 64 else 32


def part_pieces2(s1, s2, size):
    """Split a partition-range copy (dest start s1, src start s2, length
    size) into pieces legal for compute engines on both sides."""
    out = []
    off = 0
    while off < size:
        take = min(size - off, part_cap((s1 + off) % 128),
                   part_cap((s2 + off) % 128))
        out.append((s1 + off, s2 + off, take))
        off += take
    return out


def head_fragments(o_lo, o_hi, base):
    """Split channel range [o_lo, o_hi) (relative to `base`) at head
    boundaries (96) and legal partition pieces. Yields
    (head, d_lo, d_hi, p_lo, p_hi) with p relative to o_lo."""
    frags = []
    g = o_lo
    while g < o_hi:
        h = (g - base) // D
        d_lo = (g - base) - h * D
        take = min(o_hi - g, D - d_lo)
        for (d0, p0, sz) in part_pieces2(d_lo, g - o_lo, take):
            frags.append((h, d0, d0 + sz, p0, p0 + sz))
        g += take
    return frags




def build(b_loc=B_LOC):
    import concourse.bass as bass  # noqa: F401
    import concourse.tile as tile
    import concourse.bacc as bacc
    from concourse import mybir

    f32 = mybir.dt.float32
    bf16 = mybir.dt.bfloat16
    AF = mybir.ActivationFunctionType
    OP = mybir.AluOpType

    n_pair = b_loc // 2

    nc = bacc.Bacc("TRN2", target_bir_lowering=False)
    x_d = nc.dram_tensor("x", [b_loc, N, C], f32, kind="ExternalInput")
    qkvw_d = nc.dram_tensor("qkv_w", [3 * C, C], f32, kind="ExternalInput")
    projw_d = nc.dram_tensor("proj_w", [C, C], f32, kind="ExternalInput")
    projb_d = nc.dram_tensor("proj_b", [C], f32, kind="ExternalInput")
    out_d = nc.dram_tensor("out", [b_loc, N, C], f32, kind="ExternalOutput")

    with tile.TileContext(nc) as tc:
        with (
            tc.tile_pool(name="const", bufs=1) as constp,
            tc.tile_pool(name="wload", bufs=3) as wloadp,
            tc.tile_pool(name="xload", bufs=3) as xloadp,
            tc.tile_pool(name="xt", bufs=2) as xtp,
            tc.tile_pool(name="qk", bufs=2) as qkp,
            tc.tile_pool(name="vsb", bufs=2) as vsbp,
            tc.tile_pool(name="psb", bufs=2) as psbp,
            tc.tile_pool(name="ao", bufs=2) as aop,
            tc.tile_pool(name="osb", bufs=2) as osbp,
            tc.tile_pool(name="tiny", bufs=2) as tinyp,
            tc.tile_pool(name="ps_qk", bufs=2, space="PSUM") as ps_qk,
            tc.tile_pool(name="ps_s", bufs=2, space="PSUM") as ps_s,
            tc.tile_pool(name="ps_o", bufs=2, space="PSUM") as ps_o,
            tc.tile_pool(name="ps_m", bufs=2, space="PSUM") as ps_m,
        ):
            # ---------------- weights prep (no PE, all XBAR) -----------
            # qkvwT[p, c, oc] = qkv_w[oc, 128c+p]  (qkv_w.T chunk layout)
            # All loads+casts first, then all XBAR transposes back-to-
            # back: each XBAR<->copy mode transition drains the DMA
            # path (~6us), so interleaving them serializes the prep.
            qkvwT = constp.tile([128, NCH, 3 * C], bf16)
            projwT = constp.tile([128, NCH, C], bf16)
            wbs = []
            for r in range(3 * C // 128):  # 18 row-chunks of qkv_w
                wl = wloadp.tile([128, C], f32, tag="wl")
                nc.sync.dma_start(wl[:], qkvw_d[r * 128:(r + 1) * 128, :])
                wb = constp.tile([128, C], bf16, name=f"wbq{r}")
                nc.vector.tensor_copy(wb[:], wl[:])
                wbs.append((qkvwT, r, wb))
            for r in range(C // 128):  # 6 row-chunks of proj_w
                wl = wloadp.tile([128, C], f32, tag="wl")
                nc.sync.dma_start(wl[:], projw_d[r * 128:(r + 1) * 128, :])
                wb = constp.tile([128, C], bf16, name=f"wbp{r}")
                nc.vector.tensor_copy(wb[:], wl[:])
                wbs.append((projwT, r, wb))
            for (dst, r, wb) in wbs:
                nc.sync.dma_start_transpose(
                    dst[:, :, r * 128:(r + 1) * 128], wb[:])
            # per-head re-layout for proj contraction (K=96):
            # projwTh[d, h, oc] = proj_w[oc, 96h+d]
            projwTh = constp.tile([96, H, C], bf16)
            for ci in range(NCH):
                for (h, d0, d1, p0, p1) in head_fragments(
                        ci * 128, (ci + 1) * 128, 0):
                    nc.vector.tensor_copy(projwTh[d0:d1, h, :],
                                          projwT[p0:p1, ci, :])

            # proj_b broadcast to [128, C]
            pb_row = constp.tile([1, C], f32)
            nc.sync.dma_start(pb_row[:], projb_d[None, :])
            pb_bcast = constp.tile([128, C], f32)
            nc.gpsimd.partition_broadcast(pb_bcast[:], pb_row[:])

            # bias vectors for the j-tile-1 exp epilogue, per query group
            # (rows are key indices 128+p): col0 img queries, col1 lang,
            # col2 aux. Built as a free-dim row, bounced through DRAM to
            # land per-partition (SBUF APs can't transpose).
            bv_row = constp.tile([1, 3, 128], f32)
            nc.vector.memset(bv_row[:], 0.0)
            nc.vector.memset(bv_row[0:1, 0, S1 - 128:S2 - 128], -BIAS)
            nc.vector.memset(bv_row[0:1, 1, S2 - 128:N - 128], -BIAS)
            nc.vector.memset(bv_row[0:1, 2, S1 - 128:N - 128], -BIAS)
            bv_d = nc.dram_tensor("bv_scratch", [3, 128], f32,
                                  kind="Internal")
            nc.sync.dma_start(bv_d[:, :], bv_row[0:1, :, :])
            bv = constp.tile([128, 3], f32)
            nc.sync.dma_start(bv[:, :], bv_d.rearrange("i p -> p i"))

            # ---------------- per-pair of batch items ----------------
            for pr in range(n_pair):
                # load + cast + XBAR-transpose x for both batch items.
                # XBARs dispatch from the scalar HWDGE queue so their
                # mode transitions never serialize the sync DMA queue.
                xT = xtp.tile([128, NCH, 2, NPAD], bf16, tag="xT")
                xbs = []
                for bi in range(2):
                    b = 2 * pr + bi
                    for nt, (noff, nsz) in enumerate(NT):
                        xf = xloadp.tile([128, C], f32, tag="xf")
                        nc.sync.dma_start(xf[:nsz],
                                          x_d[b, noff:noff + nsz, :])
                        xb = xloadp.tile([128, C], bf16, tag="xb")
                        if nt == 1:
                            nc.vector.memset(xb[96:128, :], 0.0)
                        nc.vector.tensor_copy(xb[:nsz], xf[:nsz])
                        xbs.append((bi, noff, nsz, xb))
                for (bi, noff, nsz, xb) in xbs:
                    rows = 128 if nsz == 128 else 112
                    nc.scalar.dma_start_transpose(
                        xT[:, :, bi, noff:noff + rows], xb[:rows, :])

                # q,k transposed per head [96, H, 2, N], pair-wide moving
                q_all = qkp.tile([96, H, 2, N], bf16, tag="q_all")
                k_all = qkp.tile([96, H, 2, N], bf16, tag="k_all")
                cp_i = 0
                for oi in range(2 * C // 128):  # 12 chunks of q,k channels
                    ps = ps_qk.tile([128, 2, N], f32, tag="qk")
                    for ci in range(NCH):
                        nc.tensor.matmul(
                            ps[:, :, :],
                            qkvwT[:, ci, oi * 128:(oi + 1) * 128],
                            xT[:, ci, :, 0:N],
                            start=(ci == 0), stop=(ci == NCH - 1))
                    t = (oi * 128) // C
                    dst = q_all if t == 0 else k_all
                    oq = oi - t * NCH
                    for (h, d0, d1, p0, p1) in head_fragments(
                            oq * 128, (oq + 1) * 128, 0):
                        if cp_i % 2 == 0:
                            nc.vector.tensor_copy(dst[d0:d1, h, :, :],
                                                  ps[p0:p1, :, :])
                        else:
                            nc.scalar.copy(dst[d0:d1, h, :, :],
                                           ps[p0:p1, :, :])
                        cp_i += 1
                # stash original aux-key vectors, then overwrite aux-slot
                # columns with lang key vectors (the pre-bias "copy" edit)
                k_aux = qkp.tile([96, H, 2, S2 - S1], bf16, tag="k_aux")
                nc.gpsimd.tensor_copy(k_aux[:], k_all[:, :, :, S2:N])
                nc.gpsimd.tensor_copy(k_all[:, :, :, S2:N],
                                      k_all[:, :, :, S1:S2])

                # v natural [n, H, D+1] per (batch, token-tile)
                vp = [[None, None], [None, None]]
                vap = [None, None]
                for bi in range(2):
                    for nt, (noff, nsz) in enumerate(NT):
                        vpt = vsbp.tile([128, H, D + 1], bf16,
                                        tag=f"vp{bi}{nt}")
                        for li, (f0, fsz) in enumerate(FL):
                            psv = ps_m.tile([128, 384], f32, tag="m")
                            for ci in range(NCH):
                                nc.tensor.matmul(
                                    psv[:nsz, :fsz],
                                    xT[:, ci, bi, noff:noff + nsz],
                                    qkvwT[:, ci,
                                          2 * C + f0:2 * C + f0 + fsz],
                                    start=(ci == 0), stop=(ci == NCH - 1))
                            nc.vector.tensor_copy(
                                vpt[:nsz, 4 * li:4 * li + 4, 0:D],
                                psv[:nsz, :fsz].rearrange(
                                    "p (h d) -> p h d", h=4))
                        nc.vector.memset(vpt[:nsz, :, D:D + 1], 1.0)
                        vp[bi][nt] = vpt
                    # aux-token v rows (tokens 216:236 = rows 88:108 of
                    # tile 1): misaligned for compute engines -> DMA
                    vat = vsbp.tile([S2 - S1, H, D + 1], bf16, tag="vap")
                    nc.sync.dma_start(vat[:], vp[bi][1][88:108, :, :])
                    vap[bi] = vat

                # attention, two heads at a time
                aoT = aop.tile([96, H, 2, N], bf16, tag="aoT")
                for bi in range(2):
                    for hp in range(H // 2):
                        h0 = 2 * hp
                        pe = []
                        for jt, (joff, jsz) in enumerate(NT):
                            psj = ps_s.tile([128, 2, N], f32, tag="s")
                            for hh in range(2):
                                nc.tensor.matmul(
                                    psj[:jsz, hh, :],
                                    k_all[:, h0 + hh, bi,
                                          joff:joff + jsz],
                                    q_all[:, h0 + hh, bi, :],
                                    start=True, stop=True,
                                    skip_group_check=True)
                            pet = psbp.tile([128, 2, N], bf16,
                                            tag=f"p{jt}")
                            if jt == 0:
                                nc.scalar.activation(
                                    pet[:jsz], psj[:jsz], AF.Exp,
                                    scale=SCALE)
                            else:
                                nc.scalar.activation(
                                    pet[:jsz, :, 0:S1],
                                    psj[:jsz, :, 0:S1], AF.Exp,
                                    scale=SCALE, bias=bv[:jsz, 0:1])
                                nc.scalar.activation(
                                    pet[:jsz, :, S1:S2],
                                    psj[:jsz, :, S1:S2], AF.Exp,
                                    scale=SCALE, bias=bv[:jsz, 1:2])
                                nc.scalar.activation(
                                    pet[:jsz, :, S2:N],
                                    psj[:jsz, :, S2:N], AF.Exp,
                                    scale=SCALE, bias=bv[:jsz, 2:3])
                            pe.append(pet)
                        # true aux x aux block (suppressed in main tile)
                        ps_aa = ps_s.tile([S2 - S1, 2, S2 - S1], f32,
                                          tag="s")
                        for hh in range(2):
                            nc.tensor.matmul(
                                ps_aa[:, hh, :],
                                k_aux[:, h0 + hh, bi, :],
                                q_all[:, h0 + hh, bi, S2:N],
                                start=True, stop=True,
                                skip_group_check=True)
                        p_aa = psbp.tile([S2 - S1, 2, S2 - S1], bf16,
                                         tag="paa")
                        nc.scalar.activation(p_aa[:], ps_aa[:], AF.Exp,
                                             scale=SCALE)
                        # attn @ [v | ones] -> oT [D+1, q] + denominator
                        pso = ps_o.tile([D + 1, 2, N], f32, tag="o")
                        for hh in range(2):
                            for jt, (joff, jsz) in enumerate(NT):
                                nc.tensor.matmul(
                                    pso[:, hh, :],
                                    vp[bi][jt][:jsz, h0 + hh, :],
                                    pe[jt][:jsz, hh, :],
                                    start=(jt == 0), stop=False,
                                    skip_group_check=True)
                            nc.tensor.matmul(
                                pso[:, hh, S2:N],
                                vap[bi][:, h0 + hh, :],
                                p_aa[:, hh, :],
                                start=False, stop=True,
                                skip_group_check=True)
                        # normalize both heads
                        den = tinyp.tile([1, 2, N], f32, tag="den")
                        nc.vector.tensor_copy(den[:], pso[D:D + 1, :, :])
                        r_f = tinyp.tile([1, 2, N], f32, tag="rf")
                        nc.vector.reciprocal_approx_fast(r_f[:], den[:])
                        rbc = tinyp.tile([128, 2, N], f32, tag="rbc")
                        nc.gpsimd.partition_broadcast(
                            rbc[:],
                            r_f[0:1, :, :].rearrange("p a b -> p (a b)"))
                        nc.vector.tensor_tensor(
                            aoT[:, h0:h0 + 2, bi, :], pso[0:D, :, :],
                            rbc[0:D, :, :], OP.mult)

                # proj + bias + store (contract per head, K=96)
                for bi in range(2):
                    b = 2 * pr + bi
                    for nt, (noff, nsz) in enumerate(NT):
                        osb = osbp.tile([128, C], f32, tag="osb")
                        for li, (f0, fsz) in enumerate(FL):
                            psp = ps_m.tile([128, 384], f32, tag="m")
                            for h in range(H):
                                nc.tensor.matmul(
                                    psp[:nsz, :fsz],
                                    aoT[:, h, bi, noff:noff + nsz],
                                    projwTh[:, h, f0:f0 + fsz],
                                    start=(h == 0), stop=(h == H - 1))
                            nc.vector.tensor_tensor(
                                osb[:nsz, f0:f0 + fsz], psp[:nsz, :fsz],
                                pb_bcast[:nsz, f0:f0 + fsz], OP.add)
                        nc.sync.dma_start(out_d[b, noff:noff + nsz, :],
                                          osb[:nsz])

    nc.compile()
    return nc


_NC_CACHE = {}


def _get_nc(b_loc):
    if b_loc not in _NC_CACHE:
        _NC_CACHE[b_loc] = build(b_loc)
    return _NC_CACHE[b_loc]


def _run(inputs, trace=False):
    from concourse.bass_utils import run_bass_kernel_spmd

    x = np.ascontiguousarray(np.asarray(inputs["x"], dtype=np.float32))
    qkv_w = np.ascontiguousarray(np.asarray(inputs["qkv_w"], dtype=np.float32))
    proj_w = np.ascontiguousarray(np.asarray(inputs["proj_w"], dtype=np.float32))
    proj_b = np.ascontiguousarray(np.asarray(inputs["proj_b"], dtype=np.float32))

    nc = _get_nc(B_LOC)
    in_maps = [
        {
            "x": np.ascontiguousarray(x[i * B_LOC:(i + 1) * B_LOC]),
            "qkv_w": qkv_w,
            "proj_w": proj_w,
            "proj_b": proj_b,
        }
        for i in range(N_CORES)
    ]
    res = run_bass_kernel_spmd(
        nc, in_maps, core_ids=list(range(N_CORES)), trace=trace)
    out = np.concatenate([r["out"] for r in res.results], axis=0)
    return out, res


def kernel(x, qkv_w, proj_w, proj_b):
    out, _ = _run({"x": x, "qkv_w": qkv_w, "proj_w": proj_w,
                   "proj_b": proj_b})
    return out
